# revision 1
# baseline (speedup 1.0000x reference)
"""Trainium2 Bass kernel for nn_MultiHeadAttention (GQA + RoPE + causal softmax).

Problem (hardcoded): B=4, T=2048, C=2048, n_head=16, n_kv_head=4, head_dim=128,
fp32 in/out, rope base 10000, torch-Linear style projections (x @ W.T).

Sharding: 8 cores = (4 batches) x (2 query shards). Each core handles one batch
and 1024 query rows picked as interleaved 128-row blocks (core parity 0 takes
even blocks, parity 1 odd blocks) so both cores of a batch run an identical
instruction stream (SPMD) with identical causal work. K/V are computed for the
full sequence on both cores of a pair. No collectives; host gathers outputs.

All matmuls run in float32r (TF32-like, ~1.5e-4 rel err, 4x faster than fp32).
Everything on device is laid out transposed ([feature, token]) so every matmul
operand is in its natural layout (contraction on partitions) - the host
pre-transposes x and the weights instead.

Flash-attention-style per (kv-group, head, 256-query pair-block):
  S^T[k,q] chunks = K'^T_chunk.T @ Q'^T  -> additive causal mask in PSUM ->
  exp on ScalarE -> P^T (f32r) -> denom += ones.T @ P^T (M=1 matmul) and
  O^T += V_chunk.T @ P^T -> normalize via ones-broadcast matmul + DVE mul.
"""

import sys
import math

sys.path.insert(0, "/opt/trn_rl_repo")

import numpy as np

import concourse.bacc as bacc
import concourse.mybir as mybir
import concourse.tile as tile
from concourse.bass_utils import run_bass_kernel_spmd

F32 = mybir.dt.float32
F32R = mybir.dt.float32r
AF = mybir.ActivationFunctionType

B, T, C = 4, 2048, 2048
NH, NKV, HD = 16, 4, 128
NREP = NH // NKV              # 4 q-heads per kv head
ROPE_BASE = 10000.0
R = T // 2                    # 1024 query rows per core
NCC = C // 128                # 16 contraction chunks
NQB = R // 128                # 8 local query blocks (slots) per core
NPAIR = NQB // 2              # 4 pair-blocks of 256 queries
MASK_NEG = -30000.0


def _build_nc(nrep=1):
    nc = bacc.Bacc(trn_type="TRN2", name="mha_gqa_rope")

    xT = nc.dram_tensor("xT", [C, T], F32R, kind="ExternalInput")
    xqT = nc.dram_tensor("xqT", [C, R], F32R, kind="ExternalInput")
    wqT = nc.dram_tensor("wqT", [C, C], F32R, kind="ExternalInput")
    wkT = nc.dram_tensor("wkT", [C, NKV * HD], F32R, kind="ExternalInput")
    wvT = nc.dram_tensor("wvT", [C, NKV * HD], F32R, kind="ExternalInput")
    woT = nc.dram_tensor("woT", [C, C], F32R, kind="ExternalInput")
    cosq = nc.dram_tensor("cosq", [HD, R], F32, kind="ExternalInput")
    sinq = nc.dram_tensor("sinq", [HD, R], F32, kind="ExternalInput")
    cosk = nc.dram_tensor("cosk", [HD, T], F32, kind="ExternalInput")
    sink = nc.dram_tensor("sink", [HD, T], F32, kind="ExternalInput")
    maskadd = nc.dram_tensor("maskadd", [128, NPAIR * 4 * 512], F32, kind="ExternalInput")
    ones_d = nc.dram_tensor("ones_d", [128, 128], F32R, kind="ExternalInput")
    outT = nc.dram_tensor("outT", [C, R], F32, kind="ExternalOutput")

    with tile.TileContext(nc) as tc:
        with tc.tile_pool(name="dscr", bufs=1, space="DRAM") as dscr, \
             tc.tile_pool(name="const", bufs=1) as constp:
            qscr = dscr.tile([C, R], F32R)
            yscr = dscr.tile([C, R], F32R)

            ones_s = constp.tile([128, 128], F32R)
            nc.sync.dma_start(out=ones_s[:], in_=ones_d.ap())

            for _rep in range(nrep):

                # ---------------- Stage Q: Q'^T = rope(WqT.T @ xqT) -> qscr ----
                with tc.tile_pool(name="xq", bufs=1) as xqp, \
                     tc.tile_pool(name="wq", bufs=2) as wqp, \
                     tc.tile_pool(name="qtrig", bufs=1) as qtrigp, \
                     tc.tile_pool(name="qrope", bufs=3) as qrp, \
                     tc.tile_pool(name="qpsum", bufs=2, space="PSUM") as qps:
                    cosq_s = qtrigp.tile([HD, R], F32)
                    nc.sync.dma_start(out=cosq_s[:], in_=cosq.ap())
                    sinq_s = qtrigp.tile([HD, R], F32)
                    nc.sync.dma_start(out=sinq_s[:], in_=sinq.ap())
                    xq_s = xqp.tile([128, NCC, R], F32R)
                    nc.sync.dma_start(
                        out=xq_s[:], in_=xqT.ap().rearrange("(c p) r -> p c r", p=128)
                    )
                    for qc in range(NH):  # 16 head-chunks of Q output dims
                        wq_strip = wqp.tile([128, NCC, 128], F32R, tag="wq")
                        nc.sync.dma_start(
                            out=wq_strip[:],
                            in_=wqT.ap()[:, qc * 128:(qc + 1) * 128].rearrange(
                                "(c p) m -> p c m", p=128
                            ),
                        )
                        psq = qps.tile([128, R], F32, tag="psq")
                        for c in range(NCC):
                            for rb in range(R // 512):
                                nc.tensor.matmul(
                                    psq[:, rb * 512:(rb + 1) * 512],
                                    wq_strip[:, c, :],
                                    xq_s[:, c, rb * 512:(rb + 1) * 512],
                                    start=(c == 0),
                                    stop=(c == NCC - 1),
                                )
                        # rope in halves of 512 rows
                        for rb in range(R // 512):
                            sl = slice(rb * 512, (rb + 1) * 512)
                            q0 = qrp.tile([128, 512], F32, tag="q0")
                            nc.scalar.copy(q0[:], psq[:, sl])
                            rot = qrp.tile([128, 512], F32, tag="rot")
                            nc.scalar.dma_start(out=rot[0:64, :], in_=q0[64:128, :])
                            nc.scalar.dma_start(out=rot[64:128, :], in_=q0[0:64, :])
                            t1 = qrp.tile([128, 512], F32, tag="t1")
                            nc.vector.tensor_mul(t1[:], psq[:, sl], cosq_s[:, sl])
                            nc.vector.tensor_mul(rot[:], rot[:], sinq_s[:, sl])
                            qf = qrp.tile([128, 512], F32R, tag="qf")
                            nc.vector.tensor_add(qf[:], t1[:], rot[:])
                            nc.gpsimd.dma_start(
                                out=qscr[qc * 128:(qc + 1) * 128, sl], in_=qf[:]
                            )

                # ---------------- Stage KV ---------------------------------
                with tc.tile_pool(name="kv_res", bufs=1) as kvres:
                    kT_s = kvres.tile([128, NKV, T], F32R)   # [d, g, t]
                    v_s = kvres.tile([128, T // 128, NKV * HD], F32R)  # [t%128, tchunk, vc]

                    # merged K+V pass (t-blocks of 256; xT read once)
                    with tc.tile_pool(name="ktrig", bufs=1) as ktrigp, \
                         tc.tile_pool(name="wk", bufs=1) as wkp, \
                         tc.tile_pool(name="xt", bufs=2) as xtp, \
                         tc.tile_pool(name="krope", bufs=3) as krp, \
                         tc.tile_pool(name="kpsum", bufs=3, space="PSUM") as kps, \
                         tc.tile_pool(name="vpsum", bufs=3, space="PSUM") as vps:
                        cosk_s = ktrigp.tile([HD, T], F32)
                        nc.sync.dma_start(out=cosk_s[:], in_=cosk.ap())
                        sink_s = ktrigp.tile([HD, T], F32)
                        nc.sync.dma_start(out=sink_s[:], in_=sink.ap())
                        wk_s = wkp.tile([128, NCC, NKV * HD], F32R, tag="wk")
                        nc.sync.dma_start(
                            out=wk_s[:], in_=wkT.ap().rearrange("(c p) k -> p c k", p=128)
                        )
                        wv_s = wkp.tile([128, NCC, NKV * HD], F32R, tag="wv")
                        nc.sync.dma_start(
                            out=wv_s[:], in_=wvT.ap().rearrange("(c p) k -> p c k", p=128)
                        )
                        for tb in range(T // 256):
                            xt = xtp.tile([128, NCC, 256], F32R, tag="xt")
                            nc.sync.dma_start(
                                out=xt[:],
                                in_=xT.ap()[:, tb * 256:(tb + 1) * 256].rearrange(
                                    "(c p) t -> p c t", p=128
                                ),
                            )
                            for g in range(NKV):
                                psk = kps.tile([128, 256], F32, tag="psk")
                                for c in range(NCC):
                                    nc.tensor.matmul(
                                        psk[:],
                                        wk_s[:, c, g * 128:(g + 1) * 128],
                                        xt[:, c, :],
                                        start=(c == 0),
                                        stop=(c == NCC - 1),
                                    )
                                sl = slice(tb * 256, (tb + 1) * 256)
                                k0 = krp.tile([128, 256], F32, tag="k0")
                                nc.scalar.copy(k0[:], psk[:])
                                rot = krp.tile([128, 256], F32, tag="krot")
                                nc.scalar.dma_start(out=rot[0:64, :], in_=k0[64:128, :])
                                nc.scalar.dma_start(out=rot[64:128, :], in_=k0[0:64, :])
                                t1 = krp.tile([128, 256], F32, tag="kt1")
                                nc.vector.tensor_mul(t1[:], psk[:], cosk_s[:, sl])
                                nc.vector.tensor_mul(rot[:], rot[:], sink_s[:, sl])
                                nc.vector.tensor_add(kT_s[:, g, sl], t1[:], rot[:])
                            for ti in range(2):
                                tchunk = tb * 2 + ti
                                psv = vps.tile([128, NKV * HD], F32, tag="psv")
                                for c in range(NCC):
                                    nc.tensor.matmul(
                                        psv[:],
                                        xt[:, c, ti * 128:(ti + 1) * 128],
                                        wv_s[:, c, :],
                                        start=(c == 0),
                                        stop=(c == NCC - 1),
                                    )
                                nc.scalar.copy(v_s[:, tchunk, :], psv[:])

                    # ---------------- Stage C: attention ---------------------
                    with tc.tile_pool(name="cmask", bufs=1) as cmaskp, \
                         tc.tile_pool(name="qp", bufs=3) as qpp, \
                         tc.tile_pool(name="yt", bufs=2) as ytp_pool, \
                         tc.tile_pool(name="ptile", bufs=4) as ppp, \
                         tc.tile_pool(name="small", bufs=4) as smallp, \
                         tc.tile_pool(name="spsum", bufs=3, space="PSUM") as sps, \
                         tc.tile_pool(name="opsum", bufs=2, space="PSUM") as ops, \
                         tc.tile_pool(name="dpsum", bufs=2, space="PSUM") as dps, \
                         tc.tile_pool(name="bpsum", bufs=1, space="PSUM") as bps:
                        mask_s = cmaskp.tile([128, NPAIR * 4 * 512], F32)
                        nc.sync.dma_start(out=mask_s[:], in_=maskadd.ap())
                        for jj in range(NPAIR):
                            qp = qpp.tile([128, NH, 256], F32R, tag="qp")
                            qp_flat = qp[:].rearrange("p h q -> p (h q)")
                            nc.sync.dma_start(
                                out=qp[:],
                                in_=qscr[:, jj * 256:(jj + 1) * 256].rearrange(
                                    "(h p) q -> p h q", p=128
                                ),
                            )
                            ytp = ytp_pool.tile([128, NH, 256], F32R, tag="ytp")
                            ytp_flat = ytp[:].rearrange("p h q -> p (h q)")
                            nchunks = 4 * jj + 4
                            for g in range(NKV):
                                for hp in range(NREP // 2):   # two heads per pass
                                    hh = g * NREP + hp * 2
                                    den = dps.tile([1, 512], F32, tag="den")
                                    po = ops.tile([128, 512], F32, tag="po")
                                    for cc in range(nchunks):
                                        pss = sps.tile([128, 512], F32, tag="pss")
                                        nc.tensor.matmul(
                                            pss[:],
                                            kT_s[:, g, cc * 128:(cc + 1) * 128],
                                            qp_flat[:, hh * 256:(hh + 2) * 256],
                                            start=True,
                                            stop=True,
                                        )
                                        if cc >= 4 * jj:
                                            cb = cc - 4 * jj
                                            moff = (jj * 4 + cb) * 512
                                            nc.vector.tensor_add(
                                                pss[:], pss[:], mask_s[:, moff:moff + 512]
                                            )
                                        pt = ppp.tile([128, 512], F32R, tag="pt")
                                        nc.scalar.activation(pt[:], pss[:], AF.Exp)
                                        nc.tensor.matmul(
                                            den[:],
                                            ones_s[:, 0:1],
                                            pt[:],
                                            start=(cc == 0),
                                            stop=(cc == nchunks - 1),
                                        )
                                        nc.tensor.matmul(
                                            po[:],
                                            v_s[:, cc, g * 128:(g + 1) * 128],
                                            pt[:],
                                            start=(cc == 0),
                                            stop=(cc == nchunks - 1),
                                        )
                                    rec = smallp.tile([1, 512], F32R, tag="rec")
                                    with nc.allow_low_precision(reason="f32r softmax recip"):
                                        nc.vector.reciprocal(rec[:], den[:])
                                    pb = bps.tile([128, 512], F32, tag="pb")
                                    nc.tensor.matmul(
                                        pb[:], ones_s[0:1, :], rec[:], start=True, stop=True
                                    )
                                    bs = smallp.tile([128, 512], F32, tag="bs")
                                    nc.scalar.copy(bs[:], pb[:])
                                    nc.vector.tensor_mul(
                                        ytp_flat[:, hh * 256:(hh + 2) * 256], po[:], bs[:]
                                    )
                            nc.gpsimd.dma_start(
                                out=yscr[:, jj * 256:(jj + 1) * 256].rearrange(
                                    "(h p) q -> p h q", p=128
                                ),
                                in_=ytp[:],
                            )

                # ---------------- Stage D: out^T = WoT.T @ y^T ----------------
                with tc.tile_pool(name="yts", bufs=1) as ytsp, \
                     tc.tile_pool(name="wo", bufs=3) as wop, \
                     tc.tile_pool(name="oout", bufs=3) as ooutp, \
                     tc.tile_pool(name="opsum2", bufs=3, space="PSUM") as ops2:
                    yt_s = ytsp.tile([128, NCC, R], F32R, tag="yt_s")
                    # two half-loads: the first depends only on pair-blocks 0-1,
                    # so it overlaps the tail of stage C
                    for rb in range(R // 512):
                        rsl = slice(rb * 512, (rb + 1) * 512)
                        nc.sync.dma_start(
                            out=yt_s[:, :, rsl],
                            in_=yscr[:, rsl].rearrange("(c p) r -> p c r", p=128),
                        )
                    for oc in range(NCC):
                        wo_strip = wop.tile([128, NCC, 128], F32R, tag="wo")
                        nc.sync.dma_start(
                            out=wo_strip[:],
                            in_=woT.ap()[:, oc * 128:(oc + 1) * 128].rearrange(
                                "(c p) m -> p c m", p=128
                            ),
                        )
                        pso = ops2.tile([128, R], F32, tag="pso")
                        for c in range(NCC):
                            for rb in range(R // 512):
                                nc.tensor.matmul(
                                    pso[:, rb * 512:(rb + 1) * 512],
                                    wo_strip[:, c, :],
                                    yt_s[:, c, rb * 512:(rb + 1) * 512],
                                    start=(c == 0),
                                    stop=(c == NCC - 1),
                                )
                        ot = ooutp.tile([128, R], F32, tag="ot")
                        nc.scalar.copy(ot[:], pso[:])
                        nc.gpsimd.dma_start(
                            out=outT.ap()[oc * 128:(oc + 1) * 128, :], in_=ot[:]
                        )

    nc.finalize()
    return nc


_NC_CACHE = None


def get_nc():
    global _NC_CACHE
    if _NC_CACHE is None:
        _NC_CACHE = _build_nc()
    return _NC_CACHE


def build_nrep(nrep):
    return _build_nc(nrep=nrep)


def _qpos(parity):
    """Global query row indices (length R) for a core with given parity."""
    blocks = np.arange(NQB) * 2 + parity          # global 128-blocks
    return (blocks[:, None] * 128 + np.arange(128)[None, :]).reshape(-1)


def _trig_tables(offset):
    inv_freq = 1.0 / (ROPE_BASE ** (np.arange(0, HD, 2, dtype=np.float64) / HD))
    pos = np.arange(offset, offset + T, dtype=np.float64)
    ang = pos[:, None] * inv_freq[None, :]        # [T, 64]
    cos = np.cos(ang)                              # [T, 64]
    sin = np.sin(ang)
    cosT = np.concatenate([cos, cos], axis=1).T.astype(np.float32)  # [128, T]
    sinT = np.concatenate([-sin, sin], axis=1).T.astype(np.float32)  # sign-folded
    return np.ascontiguousarray(cosT), np.ascontiguousarray(sinT)


def _mask_table(parity):
    """Additive mask [128, NPAIR*4*512]: 0 where key <= query else MASK_NEG.
    Each 256-wide block is duplicated to 512 so one DVE add covers the
    two-head-paired [128, 512] score tile."""
    qpos = _qpos(parity)
    m = np.zeros((128, NPAIR * 4 * 512), dtype=np.float32)
    ki = np.arange(128)
    for jj in range(NPAIR):
        qcols = qpos[jj * 256:(jj + 1) * 256]      # global query positions
        for cb in range(4):
            kc = 4 * jj + cb                       # global key chunk
            kpos = kc * 128 + ki                   # [128]
            blk = np.where(kpos[:, None] <= qcols[None, :], 0.0, MASK_NEG)
            base = (jj * 4 + cb) * 512
            m[:, base:base + 256] = blk
            m[:, base + 256:base + 512] = blk
    return m


def make_in_maps(x, Wq, Wk, Wv, Wo, offset):
    x = np.asarray(x, dtype=np.float32)
    Wq = np.asarray(Wq, dtype=np.float32)
    Wk = np.asarray(Wk, dtype=np.float32)
    Wv = np.asarray(Wv, dtype=np.float32)
    Wo = np.asarray(Wo, dtype=np.float32)
    offset = int(np.asarray(offset))

    scale = 1.0 / math.sqrt(HD)
    wqT = np.ascontiguousarray((Wq * scale).T)     # [C, C] (in, out)
    wkT = np.ascontiguousarray(Wk.T)               # [C, 512]
    wvT = np.ascontiguousarray(Wv.T)
    woT = np.ascontiguousarray(Wo.T)
    cosT, sinT = _trig_tables(offset)
    ones = np.ones((128, 128), dtype=np.float32)

    in_maps = []
    for core in range(8):
        b, parity = core // 2, core % 2
        qpos = _qpos(parity)
        xb = x[b]                                   # [T, C]
        in_maps.append({
            "xT": np.ascontiguousarray(xb.T),
            "xqT": np.ascontiguousarray(xb[qpos].T),
            "wqT": wqT, "wkT": wkT, "wvT": wvT, "woT": woT,
            "cosq": np.ascontiguousarray(cosT[:, qpos]),
            "sinq": np.ascontiguousarray(sinT[:, qpos]),
            "cosk": cosT, "sink": sinT,
            "maskadd": _mask_table(parity),
            "ones_d": ones,
        })
    return in_maps


def assemble_output(results):
    out = np.empty((B, T, C), dtype=np.float32)
    for core in range(8):
        b, parity = core // 2, core % 2
        out[b, _qpos(parity), :] = results[core]["outT"].T
    return out


def kernel(x, Wq, Wk, Wv, Wo, offset):
    nc = get_nc()
    in_maps = make_in_maps(x, Wq, Wk, Wv, Wo, offset)
    res = run_bass_kernel_spmd(nc, in_maps, core_ids=list(range(8)))
    return assemble_output(res.results)

